# revision 73
# baseline (speedup 1.0000x reference)
"""MixedScoreMultiHeadAttention on 8 TRN2 NeuronCores.

Sharding: data-parallel over batch B=8 (one batch element per core, no
collectives).  Per core (R=C=256, E=512, H=8, D=64, HID=128):

  1. QKV projections (bf16 matmuls; embeddings host-pretransposed to [E, S]).
  2. Per-head dot scores (K=64 matmuls, 2 heads packed via row groups).
  3. Channel-collapse via a DRAM bounce into S4 [32g+ch, pos] so the
     score-MLP runs channel-major with 4x tile_position row-packing (K=9).
  4. MLP waves (software-pipelined): W1 (4 concurrent row-tiled matmuls) ->
     per-group relu evict (ACT+DVE split, the elementwise bottleneck) ->
     W2 (4 concurrent col-tiled M=8 matmuls) -> DRAM-bounce scatter back to
     [r, (h, c)] logit tiles, half-rchunk granularity.
  5. Softmax without max-subtraction (logits are provably O(5)), mask applied
     multiplicatively after exp (fully-masked rows via +eps on the
     denominator), DMA-transpose of the weights, AV producing out^T per
     r-half, final projection per r-half -- all interleaved with the wave
     loop of the other row chunk.

The score-MLP weights are algebraically folded on the host:
  hidden = relu(concat_h[dot_h, alpha_h*cost] @ W1)
         = relu(sum_h dot_h * W1[2h,:] + cost * sum_h alpha_h W1[2h+1,:])
so the device sees a 9-channel input (8 raw-dot channels + 1 cost channel)
and an M9 [9, HID] matrix with the 1/sqrt(D) norm folded into the dot rows.
"""

import os

os.environ.setdefault("MYCRO_LOCAL_CACHE", "1")

import numpy as np
import ml_dtypes

import concourse.bass as bass
import concourse.mybir as mybir
import concourse.tile as tile
from concourse import bacc
from concourse.bass_utils import run_bass_kernel_spmd
from concourse.masks import make_identity

try:  # best-effort NTFF profiling hook (axon image lacks it by default)
    try:
        from antenv.axon_hooks import (
            get_axon_ntff_profile_hook,
            set_axon_ntff_profile_hook,
        )
    except ImportError:
        # image's antenv lacks axon_hooks -- install a shim module so
        # bass_utils' `from antenv.axon_hooks import ...` resolves
        import sys as _sys
        import types as _types

        import antenv as _antenv

        _mod = _types.ModuleType("antenv.axon_hooks")
        _hook_box = [None]
        _mod.get_axon_ntff_profile_hook = lambda: _hook_box[0]
        _mod.set_axon_ntff_profile_hook = (
            lambda h: _hook_box.__setitem__(0, h)
        )
        _sys.modules["antenv.axon_hooks"] = _mod
        _antenv.axon_hooks = _mod
        get_axon_ntff_profile_hook = _mod.get_axon_ntff_profile_hook
        set_axon_ntff_profile_hook = _mod.set_axon_ntff_profile_hook

    if get_axon_ntff_profile_hook() is None:
        from trn_agent_boot.trn_boot import _ntff_profile_via_ctypes

        set_axon_ntff_profile_hook(
            _ntff_profile_via_ctypes("/opt/axon/libaxon_pjrt.so")
        )
except Exception:
    pass

BF16 = mybir.dt.bfloat16
F16 = mybir.dt.float16
F32 = mybir.dt.float32
AF = mybir.ActivationFunctionType
ALU = mybir.AluOpType

B, R, C, E = 8, 256, 256, 512
H, D, HID = 8, 64, 128
NCORES = 8
NWAVES = 32  # 512 positions each: (2 r-rows per 32-row group) x 256 c
EXP_ON_EVICT = os.environ.get("EXP_ON_EVICT", "1") == "1"

LAST_EXEC_NS = None
_CACHE = {}


def _build():
    nc = bacc.Bacc(
        "TRN2", target_bir_lowering=False, debug=False, enable_asserts=False
    )
    t = {}
    t["rembT"] = nc.dram_tensor("rembT", [E, R], BF16, kind="ExternalInput")
    t["cembT"] = nc.dram_tensor("cembT", [E, C], BF16, kind="ExternalInput")
    t["cost"] = nc.dram_tensor("cost16", [R, C], BF16, kind="ExternalInput")
    t["keep"] = nc.dram_tensor("keep16", [R, C], F16, kind="ExternalInput")
    for w in ("wq", "wk", "wv", "wo"):
        t[w] = nc.dram_tensor(w, [E, E], BF16, kind="ExternalInput")
    t["m9"] = nc.dram_tensor("m9", [128, HID], BF16, kind="ExternalInput")
    t["w2"] = nc.dram_tensor("w2", [HID, H], BF16, kind="ExternalInput")
    t["out"] = nc.dram_tensor("out", [R, E], F32, kind="ExternalOutput")
    # DRAM bounce buffers for cross-partition reshapes (DMA cannot stride
    # the SBUF partition dim; DRAM APs are unconstrained)
    t["fb"] = nc.dram_tensor("fbounce", [2, H, 128, C], BF16, kind="Internal")
    # holds the mixed logits in f16 (f16 mantissa keeps exp() error small),
    # channel-major wave layout; rows 32g+8 .. 32g+31 are junk
    t["mb"] = nc.dram_tensor("mbounce", [2, 128, 16 * 512], F16, kind="Internal")

    with tile.TileContext(nc) as tc:
        _kernel_body(tc, t)
    nc.compile()
    return nc


def _kernel_body(tc, t):
    nc = tc.nc
    with (
        tc.tile_pool(name="singles", bufs=1) as singles,
        tc.tile_pool(name="hp", bufs=3) as hpool,
        tc.tile_pool(name="msp", bufs=3) as mspool,
        tc.tile_pool(name="yp", bufs=2) as ypool,
        tc.tile_pool(name="mmps", bufs=2, space="PSUM") as mmps,
        tc.tile_pool(name="w1ps", bufs=1, space="PSUM") as w1ps,
        tc.tile_pool(name="w2ps", bufs=2, space="PSUM") as w2ps,
    ):
        # ---- weights/constants to SBUF, split per chunk so compute can
        # start as soon as the first chunks land; wo is loaded last ----
        def wtile(name):
            return singles.tile([128, 4 * E], BF16, tag=name, name=name)

        wq_sb, wk_sb, wv_sb, wo_sb = map(wtile, ("wq", "wk", "wv", "wo"))
        remb_sb = singles.tile([128, 4 * R], BF16, tag="remb")
        cemb_sb = singles.tile([128, 4 * C], BF16, tag="cemb")

        def load_chunks(sb, th, n, eng=None):
            # two batched dma_starts per tensor (256 rows each) -- issue
            # slots on the three DMA-capable sequencers are the scarce
            # resource in the front phase
            for k2 in range(2):
                src = (
                    th.ap()[256 * k2 : 256 * (k2 + 1), :]
                    .rearrange("(k p) c -> k p c", p=128)
                    .transpose([1, 0, 2])
                )
                (eng or nc.sync).dma_start(
                    out=sb[:, 2 * n * k2 : 2 * n * (k2 + 1)].rearrange(
                        "p (k c) -> p k c", k=2
                    ),
                    in_=src,
                )

        # spread load issue across sync/scalar/gpsimd queues -- the HWDGE
        # dma_start occupies its sequencer ~1us each
        load_chunks(remb_sb, t["rembT"], R)
        load_chunks(wq_sb, t["wq"], E, nc.scalar)
        load_chunks(cemb_sb, t["cembT"], C)
        load_chunks(wk_sb, t["wk"], E, nc.scalar)
        load_chunks(wv_sb, t["wv"], E, nc.gpsimd)
        m9_sb = singles.tile([128, HID], BF16, tag="m9")
        nc.gpsimd.dma_start(out=m9_sb, in_=t["m9"].ap())
        w2_sb = singles.tile([HID, H], BF16, tag="w2")
        nc.gpsimd.dma_start(out=w2_sb, in_=t["w2"].ap())
        keep_sb = singles.tile([128, 2, C], F16, tag="keep")
        nc.gpsimd.dma_start(
            out=keep_sb, in_=t["keep"].ap().rearrange("(i p) c -> p i c", p=128)
        )
        ident = singles.tile([128, 128], BF16, tag="ident")
        make_identity(nc, ident)
        # preload the exp table set (~2.7us) while the front phase runs so
        # the first wave's fused exp evict doesn't eat the load
        warm = singles.tile([128, 8], F32, tag="warm")
        nc.scalar.activation(out=warm, in_=ident[:, 0:8], func=AF.Exp)

        # ---- QKV projections ----
        qt_sb = singles.tile([128, 4 * R], BF16, tag="qt")  # [hd, r]
        kt_sb = singles.tile([128, 4 * C], BF16, tag="kt")  # [hd, c]
        v_sb = singles.tile([128, 2 * E], BF16, tag="v")    # [c, hd]
        f_sb = [
            singles.tile([128, 8 * C], BF16, tag=f"f{i}", name=f"f{i}")
            for i in range(2)
        ]
        s4 = [
            singles.tile([128, 8192], BF16, tag=f"s4_{i}", name=f"s4_{i}")
            for i in range(2)
        ]
        # the cost rows of S4 depend only on the input, so issue them
        # immediately (off the critical fb-gather path)
        for m in range(2):
            for g in range(4):
                nc.gpsimd.dma_start(
                    out=s4[m][32 * g + 8 : 32 * g + 9, :],
                    in_=t["cost"].ap()[
                        128 * m + 32 * g : 128 * m + 32 * (g + 1), :
                    ],
                )

        for m in range(4):  # hd chunk
            ps = mmps.tile([128, 512], F32, tag="mm")
            for k in range(4):
                nc.tensor.matmul(
                    ps[:, 0:R],
                    lhsT=wq_sb[:, 512 * k + 128 * m : 512 * k + 128 * (m + 1)],
                    rhs=remb_sb[:, R * k : R * (k + 1)],
                    start=(k == 0), stop=(k == 3),
                )
            if m % 2 == 0:
                nc.scalar.copy(out=qt_sb[:, R * m : R * (m + 1)], in_=ps[:, 0:R])
            else:
                nc.vector.tensor_copy(
                    out=qt_sb[:, R * m : R * (m + 1)], in_=ps[:, 0:R]
                )
        for m in range(4):
            ps = mmps.tile([128, 512], F32, tag="mm")
            for k in range(4):
                nc.tensor.matmul(
                    ps[:, 0:C],
                    lhsT=wk_sb[:, 512 * k + 128 * m : 512 * k + 128 * (m + 1)],
                    rhs=cemb_sb[:, C * k : C * (k + 1)],
                    start=(k == 0), stop=(k == 3),
                )
            if m % 2 == 0:
                nc.vector.tensor_copy(
                    out=kt_sb[:, C * m : C * (m + 1)], in_=ps[:, 0:C]
                )
            else:
                nc.scalar.copy(out=kt_sb[:, C * m : C * (m + 1)], in_=ps[:, 0:C])

        # ---- dot scores -> F -> DRAM bounce -> S4 (m-outer so rchunk 0's
        # collapse overlaps rchunk 1's dots) ----
        # S4[32g+ch, 8192*i + r''*256 + c] = feat_ch[128*i + 32*g + r'', c]
        for m in range(2):  # r chunk
            for j in range(4):       # qt/kt chunk (2 heads)
                for s in range(2):   # head within chunk
                    h = 2 * j + s
                    ps = mmps.tile([128, 256], F32, tag="mm")
                    nc.tensor.matmul(
                        ps,
                        lhsT=qt_sb[64 * s : 64 * (s + 1),
                                   R * j + 128 * m : R * j + 128 * (m + 1)],
                        rhs=kt_sb[64 * s : 64 * (s + 1), C * j : C * (j + 1)],
                        start=True, stop=True,
                        tile_position=(64 * s, 0),
                    )
                    if h % 2 == 0:
                        nc.scalar.copy(
                            out=f_sb[m][:, C * h : C * (h + 1)], in_=ps
                        )
                    else:
                        nc.vector.tensor_copy(
                            out=f_sb[m][:, C * h : C * (h + 1)], in_=ps
                        )
            # dump F channel-major: fb[m][ch, r_loc, c], then gather to S4;
            # gathers are split into column halves, first-half gathers for
            # all four groups issued first so wave 0 can start early
            geng = nc.sync if m == 0 else nc.scalar
            geng.dma_start(
                out=t["fb"].ap()[m].transpose([1, 0, 2]),
                in_=f_sb[m].rearrange("p (ch c) -> p ch c", ch=8),
            )
            for g in range(4):
                geng.dma_start(
                    out=s4[m][32 * g : 32 * g + 8, :].rearrange(
                        "p (a b) -> p a b", a=32
                    ),
                    in_=t["fb"].ap()[m][:, 32 * g : 32 * (g + 1), :],
                )

        # V projection is only needed from the first AV (wave ~23), so it
        # runs after the dots have cleared the critical path
        for cc in range(2):
            ps = mmps.tile([128, 512], F32, tag="mm")
            for k in range(4):
                nc.tensor.matmul(
                    ps,
                    lhsT=cemb_sb[:, C * k + 128 * cc : C * k + 128 * (cc + 1)],
                    rhs=wv_sb[:, 512 * k : 512 * (k + 1)],
                    start=(k == 0), stop=(k == 3),
                )
            nc.vector.tensor_copy(out=v_sb[:, 512 * cc : 512 * (cc + 1)], in_=ps)

        # ---- MLP waves (SW-pipelined) + interleaved softmax/AV/proj ----
        # l_sb holds the mixed logits [r, (h, c)] in f16; exp runs in
        # phase_c where all 128 partitions carry data (16x fewer
        # ACT-columns than exp'ing the sparse 32-row wave layout)
        l_sb = [
            singles.tile([128, H * C], F16, tag=f"l{i}", name=f"l{i}")
            for i in range(2)
        ]
        pt_sb = [
            singles.tile([128, H * R], BF16, tag=f"pt{cc}", name=f"pt{cc}")
            for cc in range(2)
        ]
        ot_sb = singles.tile([128, 4 * R], BF16, tag="ot")  # [e, r]

        def expevict(m):
            # evict wave m's mixed scores from PSUM (f16 logits via DVE so
            # ACT only carries the relu share), then scatter to the DRAM
            # bounce; emitted one wave later so no queue stalls on W2(m)
            i, np_ = m // 16, m % 16
            w2p = wave_w2p[m]
            ms = mspool.tile([128, 512], F16, tag="ms", name=f"ms{m}")
            if EXP_ON_EVICT:
                nc.scalar.activation(out=ms, in_=w2p, func=AF.Exp)
            else:
                nc.vector.tensor_copy(out=ms, in_=w2p)
            eng = nc.sync if i == 0 else nc.gpsimd
            eng.dma_start(
                out=t["mb"].ap()[i][:, 512 * np_ : 512 * (np_ + 1)],
                in_=ms,
            )
            if np_ % 8 == 7:
                # half-rchunk gather into l_sb [r, (h, c)]
                q = (np_ // 8) % 2
                qs = slice(4096 * q, 4096 * (q + 1))
                for g in range(4):
                    src = (
                        t["mb"].ap()[i][32 * g : 32 * g + H][:, qs]
                        .rearrange("hh (nn rp c) -> hh nn rp c", nn=8, rp=2)
                        .transpose([1, 2, 0, 3])
                    )
                    dst = l_sb[i][
                        32 * g + 16 * q : 32 * g + 16 * (q + 1), :
                    ].rearrange("p (hh c) -> p hh c", hh=H)
                    eng.dma_start(out=dst, in_=src)

        def evict_stage(n):
            wpa, wpd, ha, hd = wave_state[n]
            nc.scalar.activation(out=ha, in_=wpa, func=AF.Relu)
            nc.vector.tensor_scalar_max(out=hd, in0=wpd, scalar1=0.0)

        def w2_stage(n):
            i, np_ = n // 16, n % 16
            wpa, wpd, ha, hd = wave_state[n]
            w2p = w2ps.tile([128, 512], F32, tag="w2", name=f"w2p{n}")
            wave_w2p[n] = w2p
            for g in range(4):
                rhs = (ha if g < 2 else hd)[
                    :, 512 * (g % 2) : 512 * (g % 2 + 1)
                ]
                nc.tensor.matmul(
                    w2p[32 * g : 32 * g + 8, :],
                    lhsT=w2_sb,
                    rhs=rhs,
                    start=True, stop=True,
                    tile_position=(0, 32 * g),
                )
            if n > 0:
                expevict(n - 1)

        pc_state = {}

        def pc_tiles(i):
            if i not in pc_state:
                pc_state[i] = (
                    singles.tile([128, H], F32, tag=f"sums{i}",
                                 name=f"sums{i}"),
                    singles.tile([128, H], F32, tag=f"recips{i}",
                                 name=f"recips{i}"),
                    singles.tile([128, H * C], BF16, tag=f"pb{i}",
                                 name=f"pb{i}"),
                )
            return pc_state[i]

        def phase_c_sums(i, hh):
            # keep-mul with fused row-sum for one head (exp was fused into
            # the w2p evict)
            sums, recips, pb = pc_tiles(i)
            hs = slice(C * hh, C * (hh + 1))
            if not EXP_ON_EVICT:
                nc.scalar.activation(out=l_sb[i][:, hs], in_=l_sb[i][:, hs],
                                     func=AF.Exp)
            nc.vector.scalar_tensor_tensor(
                out=pb[:, hs],
                in0=l_sb[i][:, hs],
                scalar=1.0,
                in1=keep_sb[:, i, :],
                op0=ALU.mult,
                op1=ALU.mult,
                accum_out=sums[:, hh : hh + 1],
            )
            if hh == H - 1:
                # batched eps-add + reciprocal for all 8 heads at once
                nc.vector.tensor_scalar_add(out=sums, in0=sums, scalar1=1e-30)
                nc.vector.reciprocal(out=recips, in_=sums)

        def phase_c_av(i, hh):
            # normalize one head's weights, transpose, and AV on odd hh
            sums, recips, pb = pc_tiles(i)
            hs = slice(C * hh, C * (hh + 1))
            if hh % 2 == 0:
                nc.scalar.activation(
                    out=pb[:, hs], in_=pb[:, hs], func=AF.Copy,
                    scale=recips[:, hh : hh + 1],
                )
            else:
                nc.vector.tensor_scalar_mul(
                    out=pb[:, hs], in0=pb[:, hs],
                    scalar1=recips[:, hh : hh + 1],
                )
            for cc in range(2):
                tp = mmps.tile([128, 128], BF16, tag="mm",
                               name=f"tp{i}_{hh}_{cc}")
                nc.tensor.transpose(
                    tp,
                    in_=pb[:, C * hh + 128 * cc : C * hh + 128 * (cc + 1)],
                    identity=ident,
                )
                dstp = pt_sb[cc][:, R * hh + 128 * i : R * hh + 128 * (i + 1)]
                if (hh + cc) % 2 == 0:
                    nc.scalar.copy(out=dstp, in_=tp)
                else:
                    nc.vector.tensor_copy(out=dstp, in_=tp)
            if hh % 2 == 1:
                # AV for head pair (hh-1, hh), r-half i
                j = hh // 2
                ps = mmps.tile([128, 128], F32, tag="mm", name=f"av{i}_{j}")
                for s in range(2):
                    h = 2 * j + s
                    for cc in range(2):
                        nc.tensor.matmul(
                            ps[64 * s : 64 * (s + 1), :],
                            lhsT=v_sb[:, 512 * cc + 64 * h :
                                      512 * cc + 64 * (h + 1)],
                            rhs=pt_sb[cc][:, R * h + 128 * i :
                                          R * h + 128 * (i + 1)],
                            start=(cc == 0), stop=(cc == 1),
                        )
                if j % 2 == 0:
                    nc.vector.tensor_copy(
                        out=ot_sb[:, R * j + 128 * i : R * j + 128 * (i + 1)],
                        in_=ps,
                    )
                else:
                    nc.scalar.copy(
                        out=ot_sb[:, R * j + 128 * i : R * j + 128 * (i + 1)],
                        in_=ps,
                    )

        def tail(i):
            # output projection for r-half i
            ps = mmps.tile([128, 512], F32, tag="mm", name=f"yps{i}")
            for k in range(4):
                nc.tensor.matmul(
                    ps,
                    lhsT=ot_sb[:, R * k + 128 * i : R * k + 128 * (i + 1)],
                    rhs=wo_sb[:, 512 * k : 512 * (k + 1)],
                    start=(k == 0), stop=(k == 3),
                )
            y = ypool.tile([128, 512], F32, tag="y", name=f"y{i}")
            nc.scalar.copy(out=y, in_=ps)
            nc.sync.dma_start(out=t["out"].ap()[128 * i : 128 * (i + 1), :], in_=y)

        load_chunks(wo_sb, t["wo"], E, nc.gpsimd)

        wave_state = {}
        wave_w2p = {}

        def w1wave(n):
            i, np_ = n // 16, n % 16
            # split W1's PSUM output (and the h output) into independent
            # per-engine tiles -- a single shared tile serializes the ACT
            # and DVE evict halves through Tile's dependency tracking.
            # The DVE-side PSUM is double-buffered so W1(n+1)'s g2/g3 MMs
            # never wait on MAX(n); the ACT side relies on RELU finishing
            # first (it's the shorter evict).
            wpa = w1ps.tile([128, 1024], F32, tag="w1a", name=f"wpa{n}")
            wpd = w1ps.tile([128, 1024], F32, tag="w1d", name=f"wpd{n}")
            for g in range(4):
                dst = (wpa if g < 2 else wpd)[
                    :, 512 * (g % 2) : 512 * (g % 2 + 1)
                ]
                nc.tensor.matmul(
                    dst,
                    lhsT=m9_sb[32 * g : 32 * g + 9, :],
                    rhs=s4[i][32 * g : 32 * g + 9, 512 * np_ : 512 * (np_ + 1)],
                    start=True, stop=True,
                    tile_position=(32 * g, 0),
                )
            wave_state[n] = (
                wpa,
                wpd,
                hpool.tile([128, 1024], BF16, tag="ha", name=f"ha{n}"),
                hpool.tile([128, 1024], BF16, tag="hd", name=f"hd{n}"),
            )

        # emission order per iteration: evict(n-1) -> W1(n) -> W2(n-1) so
        # the strict PE FIFO runs W1(n) as soon as the evict frees its
        # PSUM, with W2(n-1) (whose consumer is a wave behind) after;
        # phase_c(0) is spread one head per wave once l_sb[0] has landed
        w1wave(0)
        for n in range(1, NWAVES):
            evict_stage(n - 1)
            w1wave(n)
            w2_stage(n - 1)
            # phase_c(0): 2 sum-heads per wave (19-22), then 2 AV-heads
            # per wave (23-26)
            if 19 <= n <= 22:
                phase_c_sums(0, 2 * (n - 19))
                phase_c_sums(0, 2 * (n - 19) + 1)
            if 23 <= n <= 26:
                phase_c_av(0, 2 * (n - 23))
                phase_c_av(0, 2 * (n - 23) + 1)
            if n == 27:
                tail(0)
        evict_stage(NWAVES - 1)
        w2_stage(NWAVES - 1)
        expevict(NWAVES - 1)
        for hh in range(H):
            phase_c_sums(1, hh)
        for hh in range(H):
            phase_c_av(1, hh)
        tail(1)


def _prep_inputs(row_emb, col_emb, cost_mat, attn_mask, Wq, Wk, Wv, Wo, W1,
                 W2, alpha):
    bf = ml_dtypes.bfloat16
    alpha_v = np.asarray(alpha, np.float32).reshape(-1)  # [H]
    W1 = np.asarray(W1, np.float32)
    # M9 row h (h<8): W1[2h,:]/sqrt(D); row 8: sum_h alpha_h * W1[2h+1,:]
    m9 = np.zeros((128, HID), np.float32)
    for g in range(4):
        for hh in range(H):
            m9[32 * g + hh] = W1[2 * hh] / np.sqrt(D)
        m9[32 * g + 8] = sum(alpha_v[hh] * W1[2 * hh + 1] for hh in range(H))
    shared = {
        "wq": np.asarray(Wq, np.float32).astype(bf),
        "wk": np.asarray(Wk, np.float32).astype(bf),
        "wv": np.asarray(Wv, np.float32).astype(bf),
        "wo": np.asarray(Wo, np.float32).astype(bf),
        "m9": m9.astype(bf),
        "w2": np.asarray(W2, np.float32).astype(bf),
    }
    in_maps = []
    for b in range(B):
        m = dict(shared)
        m["rembT"] = np.ascontiguousarray(
            np.asarray(row_emb[b], np.float32).T
        ).astype(bf)
        m["cembT"] = np.ascontiguousarray(
            np.asarray(col_emb[b], np.float32).T
        ).astype(bf)
        m["cost16"] = np.asarray(cost_mat[b, :, :, 0], np.float32).astype(bf)
        m["keep16"] = (~np.asarray(attn_mask[b])).astype(np.float16)
        in_maps.append(m)
    return in_maps


def kernel(**inputs) -> np.ndarray:
    global LAST_EXEC_NS
    if "nc" not in _CACHE:
        _CACHE["nc"] = _build()
    nc = _CACHE["nc"]
    in_maps = _prep_inputs(**inputs)
    trace = os.environ.get("KERNEL_TRACE", "0") == "1"
    res = run_bass_kernel_spmd(
        nc, in_maps, core_ids=list(range(NCORES)), trace=trace
    )
    LAST_EXEC_NS = res.exec_time_ns
    out = np.stack([np.asarray(res.results[b]["out"]) for b in range(B)])
    return out.astype(np.float32)



# revision 74
# speedup vs baseline: 1.0387x; 1.0387x over previous
"""MixedScoreMultiHeadAttention on 8 TRN2 NeuronCores.

Sharding: data-parallel over batch B=8 (one batch element per core, no
collectives).  Per core (R=C=256, E=512, H=8, D=64, HID=128):

  1. QKV projections (bf16 matmuls; embeddings host-pretransposed to [E, S]).
  2. Per-head dot scores (K=64 matmuls, 2 heads packed via row groups).
  3. Channel-collapse via a DRAM bounce into S4 [32g+ch, pos] so the
     score-MLP runs channel-major with 4x tile_position row-packing (K=9).
  4. MLP waves (software-pipelined): W1 (4 concurrent row-tiled matmuls) ->
     per-group relu evict (ACT+DVE split, the elementwise bottleneck) ->
     W2 (4 concurrent col-tiled M=8 matmuls) -> DRAM-bounce scatter back to
     [r, (h, c)] logit tiles, half-rchunk granularity.
  5. Softmax without max-subtraction (logits are provably O(5)), mask applied
     multiplicatively after exp (fully-masked rows via +eps on the
     denominator), DMA-transpose of the weights, AV producing out^T per
     r-half, final projection per r-half -- all interleaved with the wave
     loop of the other row chunk.

The score-MLP weights are algebraically folded on the host:
  hidden = relu(concat_h[dot_h, alpha_h*cost] @ W1)
         = relu(sum_h dot_h * W1[2h,:] + cost * sum_h alpha_h W1[2h+1,:])
so the device sees a 9-channel input (8 raw-dot channels + 1 cost channel)
and an M9 [9, HID] matrix with the 1/sqrt(D) norm folded into the dot rows.
"""

import os

os.environ.setdefault("MYCRO_LOCAL_CACHE", "1")

import numpy as np
import ml_dtypes

import concourse.bass as bass
import concourse.mybir as mybir
import concourse.tile as tile
from concourse import bacc
from concourse.bass_utils import run_bass_kernel_spmd
from concourse.masks import make_identity

try:  # best-effort NTFF profiling hook (axon image lacks it by default)
    try:
        from antenv.axon_hooks import (
            get_axon_ntff_profile_hook,
            set_axon_ntff_profile_hook,
        )
    except ImportError:
        # image's antenv lacks axon_hooks -- install a shim module so
        # bass_utils' `from antenv.axon_hooks import ...` resolves
        import sys as _sys
        import types as _types

        import antenv as _antenv

        _mod = _types.ModuleType("antenv.axon_hooks")
        _hook_box = [None]
        _mod.get_axon_ntff_profile_hook = lambda: _hook_box[0]
        _mod.set_axon_ntff_profile_hook = (
            lambda h: _hook_box.__setitem__(0, h)
        )
        _sys.modules["antenv.axon_hooks"] = _mod
        _antenv.axon_hooks = _mod
        get_axon_ntff_profile_hook = _mod.get_axon_ntff_profile_hook
        set_axon_ntff_profile_hook = _mod.set_axon_ntff_profile_hook

    if get_axon_ntff_profile_hook() is None:
        from trn_agent_boot.trn_boot import _ntff_profile_via_ctypes

        set_axon_ntff_profile_hook(
            _ntff_profile_via_ctypes("/opt/axon/libaxon_pjrt.so")
        )
except Exception:
    pass

BF16 = mybir.dt.bfloat16
F16 = mybir.dt.float16
F32 = mybir.dt.float32
AF = mybir.ActivationFunctionType
ALU = mybir.AluOpType

B, R, C, E = 8, 256, 256, 512
H, D, HID = 8, 64, 128
NCORES = 8
NWAVES = 32  # 512 positions each: (2 r-rows per 32-row group) x 256 c
EXP_ON_EVICT = os.environ.get("EXP_ON_EVICT", "1") == "1"

LAST_EXEC_NS = None
_CACHE = {}


def _build():
    nc = bacc.Bacc(
        "TRN2", target_bir_lowering=False, debug=False, enable_asserts=False
    )
    t = {}
    t["rembT"] = nc.dram_tensor("rembT", [E, R], BF16, kind="ExternalInput")
    t["cembT"] = nc.dram_tensor("cembT", [E, C], BF16, kind="ExternalInput")
    t["cost"] = nc.dram_tensor("cost16", [R, C], BF16, kind="ExternalInput")
    t["keep"] = nc.dram_tensor("keep16", [R, C], F16, kind="ExternalInput")
    for w in ("wq", "wk", "wv", "wo"):
        t[w] = nc.dram_tensor(w, [E, E], BF16, kind="ExternalInput")
    t["m9"] = nc.dram_tensor("m9", [128, HID], BF16, kind="ExternalInput")
    t["w2"] = nc.dram_tensor("w2", [HID, H], BF16, kind="ExternalInput")
    t["out"] = nc.dram_tensor("out", [R, E], F32, kind="ExternalOutput")
    # DRAM bounce buffers for cross-partition reshapes (DMA cannot stride
    # the SBUF partition dim; DRAM APs are unconstrained)
    t["fb"] = nc.dram_tensor("fbounce", [2, H, 128, C], BF16, kind="Internal")
    # holds the mixed logits in f16 (f16 mantissa keeps exp() error small),
    # channel-major wave layout; rows 32g+8 .. 32g+31 are junk
    t["mb"] = nc.dram_tensor("mbounce", [2, 128, 16 * 512], F16, kind="Internal")

    with tile.TileContext(nc) as tc:
        _kernel_body(tc, t)
    nc.compile()
    return nc


def _kernel_body(tc, t):
    nc = tc.nc
    with (
        tc.tile_pool(name="singles", bufs=1) as singles,
        tc.tile_pool(name="hp", bufs=3) as hpool,
        tc.tile_pool(name="msp", bufs=3) as mspool,
        tc.tile_pool(name="yp", bufs=2) as ypool,
        tc.tile_pool(name="mmps", bufs=2, space="PSUM") as mmps,
        tc.tile_pool(name="w1ps", bufs=1, space="PSUM") as w1ps,
        tc.tile_pool(name="w2ps", bufs=2, space="PSUM") as w2ps,
    ):
        # ---- weights/constants to SBUF, split per chunk so compute can
        # start as soon as the first chunks land; wo is loaded last ----
        def wtile(name):
            return singles.tile([128, 4 * E], BF16, tag=name, name=name)

        wq_sb, wk_sb, wv_sb, wo_sb = map(wtile, ("wq", "wk", "wv", "wo"))
        remb_sb = singles.tile([128, 4 * R], BF16, tag="remb")
        cemb_sb = singles.tile([128, 4 * C], BF16, tag="cemb")

        def load_chunks(sb, th, n, eng=None):
            # two batched dma_starts per tensor (256 rows each) -- issue
            # slots on the three DMA-capable sequencers are the scarce
            # resource in the front phase
            for k2 in range(2):
                src = (
                    th.ap()[256 * k2 : 256 * (k2 + 1), :]
                    .rearrange("(k p) c -> k p c", p=128)
                    .transpose([1, 0, 2])
                )
                (eng or nc.sync).dma_start(
                    out=sb[:, 2 * n * k2 : 2 * n * (k2 + 1)].rearrange(
                        "p (k c) -> p k c", k=2
                    ),
                    in_=src,
                )

        # spread load issue across sync/scalar/gpsimd queues -- the HWDGE
        # dma_start occupies its sequencer ~1us each
        load_chunks(remb_sb, t["rembT"], R)
        load_chunks(wq_sb, t["wq"], E, nc.scalar)
        load_chunks(cemb_sb, t["cembT"], C)
        load_chunks(wk_sb, t["wk"], E, nc.scalar)
        load_chunks(wv_sb, t["wv"], E, nc.gpsimd)
        m9_sb = singles.tile([128, HID], BF16, tag="m9")
        nc.gpsimd.dma_start(out=m9_sb, in_=t["m9"].ap())
        w2_sb = singles.tile([HID, H], BF16, tag="w2")
        nc.gpsimd.dma_start(out=w2_sb, in_=t["w2"].ap())
        keep_sb = singles.tile([128, 2, C], F16, tag="keep")
        nc.gpsimd.dma_start(
            out=keep_sb, in_=t["keep"].ap().rearrange("(i p) c -> p i c", p=128)
        )
        ident = singles.tile([128, 128], BF16, tag="ident")
        make_identity(nc, ident)
        # preload the exp table set (~2.7us) while the front phase runs so
        # the first wave's fused exp evict doesn't eat the load
        warm = singles.tile([128, 8], F32, tag="warm")
        nc.scalar.activation(out=warm, in_=ident[:, 0:8], func=AF.Exp)

        # ---- QKV projections ----
        qt_sb = singles.tile([128, 4 * R], BF16, tag="qt")  # [hd, r]
        kt_sb = singles.tile([128, 4 * C], BF16, tag="kt")  # [hd, c]
        v_sb = singles.tile([128, 2 * E], BF16, tag="v")    # [c, hd]
        f_sb = [
            singles.tile([128, 8 * C], BF16, tag=f"f{i}", name=f"f{i}")
            for i in range(2)
        ]
        s4 = [
            singles.tile([128, 8192], BF16, tag=f"s4_{i}", name=f"s4_{i}")
            for i in range(2)
        ]
        # the cost rows of S4 depend only on the input, so issue them
        # immediately (off the critical fb-gather path)
        for m in range(2):
            for g in range(4):
                nc.gpsimd.dma_start(
                    out=s4[m][32 * g + 8 : 32 * g + 9, :],
                    in_=t["cost"].ap()[
                        128 * m + 32 * g : 128 * m + 32 * (g + 1), :
                    ],
                )

        for m in range(4):  # hd chunk
            ps = mmps.tile([128, 512], F32, tag="mm")
            for k in range(4):
                nc.tensor.matmul(
                    ps[:, 0:R],
                    lhsT=wq_sb[:, 512 * k + 128 * m : 512 * k + 128 * (m + 1)],
                    rhs=remb_sb[:, R * k : R * (k + 1)],
                    start=(k == 0), stop=(k == 3),
                )
            if m % 2 == 0:
                nc.scalar.copy(out=qt_sb[:, R * m : R * (m + 1)], in_=ps[:, 0:R])
            else:
                nc.vector.tensor_copy(
                    out=qt_sb[:, R * m : R * (m + 1)], in_=ps[:, 0:R]
                )
        for m in range(4):
            ps = mmps.tile([128, 512], F32, tag="mm")
            for k in range(4):
                nc.tensor.matmul(
                    ps[:, 0:C],
                    lhsT=wk_sb[:, 512 * k + 128 * m : 512 * k + 128 * (m + 1)],
                    rhs=cemb_sb[:, C * k : C * (k + 1)],
                    start=(k == 0), stop=(k == 3),
                )
            if m % 2 == 0:
                nc.vector.tensor_copy(
                    out=kt_sb[:, C * m : C * (m + 1)], in_=ps[:, 0:C]
                )
            else:
                nc.scalar.copy(out=kt_sb[:, C * m : C * (m + 1)], in_=ps[:, 0:C])

        # ---- dot scores -> F -> DRAM bounce -> S4 (m-outer so rchunk 0's
        # collapse overlaps rchunk 1's dots) ----
        # S4[32g+ch, 8192*i + r''*256 + c] = feat_ch[128*i + 32*g + r'', c]
        for m in range(2):  # r chunk
            for j in range(4):       # qt/kt chunk (2 heads)
                for s in range(2):   # head within chunk
                    h = 2 * j + s
                    ps = mmps.tile([128, 256], F32, tag="mm")
                    nc.tensor.matmul(
                        ps,
                        lhsT=qt_sb[64 * s : 64 * (s + 1),
                                   R * j + 128 * m : R * j + 128 * (m + 1)],
                        rhs=kt_sb[64 * s : 64 * (s + 1), C * j : C * (j + 1)],
                        start=True, stop=True,
                        tile_position=(64 * s, 0),
                    )
                    if h % 2 == 0:
                        nc.scalar.copy(
                            out=f_sb[m][:, C * h : C * (h + 1)], in_=ps
                        )
                    else:
                        nc.vector.tensor_copy(
                            out=f_sb[m][:, C * h : C * (h + 1)], in_=ps
                        )
            # dump F channel-major: fb[m][ch, r_loc, c], then gather to S4;
            # gathers are split into column halves, first-half gathers for
            # all four groups issued first so wave 0 can start early
            geng = nc.sync if m == 0 else nc.scalar
            for hp in range(2):
                geng.dma_start(
                    out=t["fb"].ap()[m][4 * hp : 4 * (hp + 1)].transpose(
                        [1, 0, 2]
                    ),
                    in_=f_sb[m][:, 1024 * hp : 1024 * (hp + 1)].rearrange(
                        "p (ch c) -> p ch c", ch=4
                    ),
                )
            for ch_half in range(2):
                rh = slice(16 * ch_half, 16 * (ch_half + 1))
                cs = slice(4096 * ch_half, 4096 * (ch_half + 1))
                for g in range(4):
                    geng.dma_start(
                        out=s4[m][32 * g : 32 * g + 8, cs].rearrange(
                            "p (a b) -> p a b", a=16
                        ),
                        in_=t["fb"].ap()[m][:, 32 * g + rh.start : 32 * g + rh.stop, :],
                    )

        # V projection is only needed from the first AV (wave ~23), so it
        # runs after the dots have cleared the critical path
        for cc in range(2):
            ps = mmps.tile([128, 512], F32, tag="mm")
            for k in range(4):
                nc.tensor.matmul(
                    ps,
                    lhsT=cemb_sb[:, C * k + 128 * cc : C * k + 128 * (cc + 1)],
                    rhs=wv_sb[:, 512 * k : 512 * (k + 1)],
                    start=(k == 0), stop=(k == 3),
                )
            nc.vector.tensor_copy(out=v_sb[:, 512 * cc : 512 * (cc + 1)], in_=ps)

        # ---- MLP waves (SW-pipelined) + interleaved softmax/AV/proj ----
        # l_sb holds the mixed logits [r, (h, c)] in f16; exp runs in
        # phase_c where all 128 partitions carry data (16x fewer
        # ACT-columns than exp'ing the sparse 32-row wave layout)
        l_sb = [
            singles.tile([128, H * C], F16, tag=f"l{i}", name=f"l{i}")
            for i in range(2)
        ]
        pt_sb = [
            singles.tile([128, H * R], BF16, tag=f"pt{cc}", name=f"pt{cc}")
            for cc in range(2)
        ]
        ot_sb = singles.tile([128, 4 * R], BF16, tag="ot")  # [e, r]

        def expevict(m):
            # evict wave m's mixed scores from PSUM (f16 logits via DVE so
            # ACT only carries the relu share), then scatter to the DRAM
            # bounce; emitted one wave later so no queue stalls on W2(m)
            i, np_ = m // 16, m % 16
            w2p = wave_w2p[m]
            ms = mspool.tile([128, 512], F16, tag="ms", name=f"ms{m}")
            if EXP_ON_EVICT:
                nc.scalar.activation(out=ms, in_=w2p, func=AF.Exp)
            else:
                nc.vector.tensor_copy(out=ms, in_=w2p)
            eng = nc.sync if i == 0 else nc.gpsimd
            eng.dma_start(
                out=t["mb"].ap()[i][:, 512 * np_ : 512 * (np_ + 1)],
                in_=ms,
            )
            if np_ % 8 == 7:
                # half-rchunk gather into l_sb [r, (h, c)]
                q = (np_ // 8) % 2
                qs = slice(4096 * q, 4096 * (q + 1))
                for g in range(4):
                    src = (
                        t["mb"].ap()[i][32 * g : 32 * g + H][:, qs]
                        .rearrange("hh (nn rp c) -> hh nn rp c", nn=8, rp=2)
                        .transpose([1, 2, 0, 3])
                    )
                    dst = l_sb[i][
                        32 * g + 16 * q : 32 * g + 16 * (q + 1), :
                    ].rearrange("p (hh c) -> p hh c", hh=H)
                    eng.dma_start(out=dst, in_=src)

        def evict_stage(n):
            wpa, wpd, ha, hd = wave_state[n]
            nc.scalar.activation(out=ha, in_=wpa, func=AF.Relu)
            nc.vector.tensor_scalar_max(out=hd, in0=wpd, scalar1=0.0)

        def w2_stage(n):
            i, np_ = n // 16, n % 16
            wpa, wpd, ha, hd = wave_state[n]
            w2p = w2ps.tile([128, 512], F32, tag="w2", name=f"w2p{n}")
            wave_w2p[n] = w2p
            for g in range(4):
                rhs = (ha if g < 2 else hd)[
                    :, 512 * (g % 2) : 512 * (g % 2 + 1)
                ]
                nc.tensor.matmul(
                    w2p[32 * g : 32 * g + 8, :],
                    lhsT=w2_sb,
                    rhs=rhs,
                    start=True, stop=True,
                    tile_position=(0, 32 * g),
                )
            if n > 0:
                expevict(n - 1)

        pc_state = {}

        def pc_tiles(i):
            if i not in pc_state:
                pc_state[i] = (
                    singles.tile([128, H], F32, tag=f"sums{i}",
                                 name=f"sums{i}"),
                    singles.tile([128, H], F32, tag=f"recips{i}",
                                 name=f"recips{i}"),
                    singles.tile([128, H * C], BF16, tag=f"pb{i}",
                                 name=f"pb{i}"),
                )
            return pc_state[i]

        def phase_c_sums(i, hh):
            # keep-mul with fused row-sum for one head (exp was fused into
            # the w2p evict)
            sums, recips, pb = pc_tiles(i)
            hs = slice(C * hh, C * (hh + 1))
            if not EXP_ON_EVICT:
                nc.scalar.activation(out=l_sb[i][:, hs], in_=l_sb[i][:, hs],
                                     func=AF.Exp)
            nc.vector.scalar_tensor_tensor(
                out=pb[:, hs],
                in0=l_sb[i][:, hs],
                scalar=1.0,
                in1=keep_sb[:, i, :],
                op0=ALU.mult,
                op1=ALU.mult,
                accum_out=sums[:, hh : hh + 1],
            )
            if hh == H - 1:
                # batched eps-add + reciprocal for all 8 heads at once
                nc.vector.tensor_scalar_add(out=sums, in0=sums, scalar1=1e-30)
                nc.vector.reciprocal(out=recips, in_=sums)

        def phase_c_av(i, hh):
            # normalize one head's weights, transpose, and AV on odd hh
            sums, recips, pb = pc_tiles(i)
            hs = slice(C * hh, C * (hh + 1))
            if hh % 2 == 0:
                nc.scalar.activation(
                    out=pb[:, hs], in_=pb[:, hs], func=AF.Copy,
                    scale=recips[:, hh : hh + 1],
                )
            else:
                nc.vector.tensor_scalar_mul(
                    out=pb[:, hs], in0=pb[:, hs],
                    scalar1=recips[:, hh : hh + 1],
                )
            for cc in range(2):
                tp = mmps.tile([128, 128], BF16, tag="mm",
                               name=f"tp{i}_{hh}_{cc}")
                nc.tensor.transpose(
                    tp,
                    in_=pb[:, C * hh + 128 * cc : C * hh + 128 * (cc + 1)],
                    identity=ident,
                )
                dstp = pt_sb[cc][:, R * hh + 128 * i : R * hh + 128 * (i + 1)]
                if (hh + cc) % 2 == 0:
                    nc.scalar.copy(out=dstp, in_=tp)
                else:
                    nc.vector.tensor_copy(out=dstp, in_=tp)
            if hh % 2 == 1:
                # AV for head pair (hh-1, hh), r-half i
                j = hh // 2
                ps = mmps.tile([128, 128], F32, tag="mm", name=f"av{i}_{j}")
                for s in range(2):
                    h = 2 * j + s
                    for cc in range(2):
                        nc.tensor.matmul(
                            ps[64 * s : 64 * (s + 1), :],
                            lhsT=v_sb[:, 512 * cc + 64 * h :
                                      512 * cc + 64 * (h + 1)],
                            rhs=pt_sb[cc][:, R * h + 128 * i :
                                          R * h + 128 * (i + 1)],
                            start=(cc == 0), stop=(cc == 1),
                        )
                if j % 2 == 0:
                    nc.vector.tensor_copy(
                        out=ot_sb[:, R * j + 128 * i : R * j + 128 * (i + 1)],
                        in_=ps,
                    )
                else:
                    nc.scalar.copy(
                        out=ot_sb[:, R * j + 128 * i : R * j + 128 * (i + 1)],
                        in_=ps,
                    )

        def tail(i):
            # output projection for r-half i
            ps = mmps.tile([128, 512], F32, tag="mm", name=f"yps{i}")
            for k in range(4):
                nc.tensor.matmul(
                    ps,
                    lhsT=ot_sb[:, R * k + 128 * i : R * k + 128 * (i + 1)],
                    rhs=wo_sb[:, 512 * k : 512 * (k + 1)],
                    start=(k == 0), stop=(k == 3),
                )
            y = ypool.tile([128, 512], F32, tag="y", name=f"y{i}")
            nc.scalar.copy(out=y, in_=ps)
            nc.sync.dma_start(out=t["out"].ap()[128 * i : 128 * (i + 1), :], in_=y)

        load_chunks(wo_sb, t["wo"], E, nc.gpsimd)

        wave_state = {}
        wave_w2p = {}

        def w1wave(n):
            i, np_ = n // 16, n % 16
            # split W1's PSUM output (and the h output) into independent
            # per-engine tiles -- a single shared tile serializes the ACT
            # and DVE evict halves through Tile's dependency tracking.
            # The DVE-side PSUM is double-buffered so W1(n+1)'s g2/g3 MMs
            # never wait on MAX(n); the ACT side relies on RELU finishing
            # first (it's the shorter evict).
            wpa = w1ps.tile([128, 1024], F32, tag="w1a", name=f"wpa{n}")
            wpd = w1ps.tile([128, 1024], F32, tag="w1d", name=f"wpd{n}")
            for g in range(4):
                dst = (wpa if g < 2 else wpd)[
                    :, 512 * (g % 2) : 512 * (g % 2 + 1)
                ]
                nc.tensor.matmul(
                    dst,
                    lhsT=m9_sb[32 * g : 32 * g + 9, :],
                    rhs=s4[i][32 * g : 32 * g + 9, 512 * np_ : 512 * (np_ + 1)],
                    start=True, stop=True,
                    tile_position=(32 * g, 0),
                )
            wave_state[n] = (
                wpa,
                wpd,
                hpool.tile([128, 1024], BF16, tag="ha", name=f"ha{n}"),
                hpool.tile([128, 1024], BF16, tag="hd", name=f"hd{n}"),
            )

        # emission order per iteration: evict(n-1) -> W1(n) -> W2(n-1) so
        # the strict PE FIFO runs W1(n) as soon as the evict frees its
        # PSUM, with W2(n-1) (whose consumer is a wave behind) after;
        # phase_c(0) is spread one head per wave once l_sb[0] has landed
        w1wave(0)
        for n in range(1, NWAVES):
            evict_stage(n - 1)
            w1wave(n)
            w2_stage(n - 1)
            # phase_c(0): 2 sum-heads per wave (19-22), then 2 AV-heads
            # per wave (23-26)
            if 19 <= n <= 22:
                phase_c_sums(0, 2 * (n - 19))
                phase_c_sums(0, 2 * (n - 19) + 1)
            if 23 <= n <= 26:
                phase_c_av(0, 2 * (n - 23))
                phase_c_av(0, 2 * (n - 23) + 1)
            if n == 27:
                tail(0)
        evict_stage(NWAVES - 1)
        w2_stage(NWAVES - 1)
        expevict(NWAVES - 1)
        for hh in range(H):
            phase_c_sums(1, hh)
        for hh in range(H):
            phase_c_av(1, hh)
        tail(1)


def _prep_inputs(row_emb, col_emb, cost_mat, attn_mask, Wq, Wk, Wv, Wo, W1,
                 W2, alpha):
    bf = ml_dtypes.bfloat16
    alpha_v = np.asarray(alpha, np.float32).reshape(-1)  # [H]
    W1 = np.asarray(W1, np.float32)
    # M9 row h (h<8): W1[2h,:]/sqrt(D); row 8: sum_h alpha_h * W1[2h+1,:]
    m9 = np.zeros((128, HID), np.float32)
    for g in range(4):
        for hh in range(H):
            m9[32 * g + hh] = W1[2 * hh] / np.sqrt(D)
        m9[32 * g + 8] = sum(alpha_v[hh] * W1[2 * hh + 1] for hh in range(H))
    shared = {
        "wq": np.asarray(Wq, np.float32).astype(bf),
        "wk": np.asarray(Wk, np.float32).astype(bf),
        "wv": np.asarray(Wv, np.float32).astype(bf),
        "wo": np.asarray(Wo, np.float32).astype(bf),
        "m9": m9.astype(bf),
        "w2": np.asarray(W2, np.float32).astype(bf),
    }
    in_maps = []
    for b in range(B):
        m = dict(shared)
        m["rembT"] = np.ascontiguousarray(
            np.asarray(row_emb[b], np.float32).T
        ).astype(bf)
        m["cembT"] = np.ascontiguousarray(
            np.asarray(col_emb[b], np.float32).T
        ).astype(bf)
        m["cost16"] = np.asarray(cost_mat[b, :, :, 0], np.float32).astype(bf)
        m["keep16"] = (~np.asarray(attn_mask[b])).astype(np.float16)
        in_maps.append(m)
    return in_maps


def kernel(**inputs) -> np.ndarray:
    global LAST_EXEC_NS
    if "nc" not in _CACHE:
        _CACHE["nc"] = _build()
    nc = _CACHE["nc"]
    in_maps = _prep_inputs(**inputs)
    trace = os.environ.get("KERNEL_TRACE", "0") == "1"
    res = run_bass_kernel_spmd(
        nc, in_maps, core_ids=list(range(NCORES)), trace=trace
    )
    LAST_EXEC_NS = res.exec_time_ns
    out = np.stack([np.asarray(res.results[b]["out"]) for b in range(B)])
    return out.astype(np.float32)

